# revision 49
# baseline (speedup 1.0000x reference)
"""Multi-head attention (BN-folded QKV + rel-pos bias + GELU + out-proj) on 8 TRN2 cores.

Data-parallel over batch (b=8 -> 1 batch element per core, no collectives).

All BatchNorms are eval-mode affine transforms folded into the projection
weights/biases on the host.  The additive Toeplitz position bias is folded in
multiplicatively after exp:  exp(dots + E) = exp(dots) * exp(E), with exp(E)
shipped as per-partition pre-shifted window tiles so each [j, i] tile of the
attention matrix multiplies a plain strided SBUF view.

Steady state is ACT-bound (64 back-to-back [128,1024] exps at ~1us; nothing
else on the chip evaluates exp), so everything is arranged to keep the scalar
engine fed without gaps:
 - inputs split across the SP / ACT / GpSimd DMA queues, win tile loaded in
   three column chunks ordered by jc consumption;
 - dummy matmuls during the DMA wait pre-release the PE HAM clock gate
   (1.2 -> 2.4 GHz) so the projection + first dots run warm;
 - attn@v drains and the norm chains are emitted at each pair's last unit, so
   the next pair's PSUM accumulators never WAR-stall on a late norm (the
   baseline lost ~1.5-2us + a HAM re-throttle per pair boundary there);
 - v-bias is applied as the gelu's per-partition bias instead of rank-1
   matmuls, and the ones-columns span 64 lanes so the softmax reciprocal
   needs no SBUF->SBUF widen hop.
"""

import numpy as np
import ml_dtypes

HEADS, DK, DV = 8, 32, 64
DIM, N, DIM_OUT = 256, 1024, 256
IDK, IDV = DK * HEADS, DV * HEADS  # 256, 512
SCALE = DK ** -0.5
EPS = 1e-5
B = 8
WIN = 1920  # window tile width: covers i - 128*jc in [-896, 1023]

BF16 = ml_dtypes.bfloat16


def _prep_host(x, Wq, Wk, Wv, Wo, bo, pos_emb,
               q_gamma, q_beta, q_mean, q_var,
               k_gamma, k_beta, k_mean, k_var,
               v_gamma, v_beta, v_mean, v_var,
               o_gamma, o_beta, o_mean, o_var):
    f32 = np.float32
    inv_q = (q_gamma / np.sqrt(q_var + EPS)).astype(f32)
    inv_k = (k_gamma / np.sqrt(k_var + EPS)).astype(f32)
    inv_v = (v_gamma / np.sqrt(v_var + EPS)).astype(f32)
    inv_o = (o_gamma / np.sqrt(o_var + EPS)).astype(f32)

    # q also absorbs the attention scale
    Wq_eff = (Wq * inv_q[:, None]) * SCALE
    bq = ((q_beta - q_mean * inv_q) * SCALE).astype(f32)
    Wk_eff = Wk * inv_k[:, None]
    bk = (k_beta - k_mean * inv_k).astype(f32)
    Wv_eff = Wv * inv_v[:, None]
    bv = (v_beta - v_mean * inv_v).astype(f32)
    Wo_eff = Wo * inv_o[:, None]
    bo_eff = ((bo - o_mean) * inv_o + o_beta).astype(f32)

    # lhsT layouts, pre-chunked to the exact SBUF tile shapes
    def chunk_T(w, kchunks):  # [O, C] -> [128, kchunks, O]  (WT[c, o] tiled)
        wT = np.ascontiguousarray(w.T.astype(f32))  # [C, O]
        c, o = wT.shape
        assert c == kchunks * 128
        return np.ascontiguousarray(
            wT.reshape(kchunks, 128, o).transpose(1, 0, 2)).astype(BF16)

    wqs = chunk_T(Wq_eff, 2)            # [128, 2, 256]
    wks = chunk_T(Wk_eff, 2)            # [128, 2, 256]
    wvs = chunk_T(Wv_eff, 2)            # [128, 2, 512]
    wos = chunk_T(Wo_eff, 4)            # [128, 4, 256]

    def chunk_bias(b, chunks):  # [C] -> [128, chunks]
        return np.ascontiguousarray(
            b.reshape(chunks, 128).T).astype(f32)

    bqs = chunk_bias(bq, 2)             # [128, 2]
    bks = chunk_bias(bk, 2)             # [128, 2]
    bos = chunk_bias(bo_eff, 2)         # [128, 2]

    # v-bias applied as the gelu's per-partition bias: g_sb partition p of
    # chunk c holds head 2c + (p>=64), channel dv = p%64.
    # gelu(out_u/sums + bv) == gelu((attn@(v+bv))/sums) since sums normalize.
    bvg = np.empty((128, 4), dtype=f32)
    for c in range(4):
        for p in range(128):
            h = 2 * c + (1 if p >= DV else 0)
            bvg[p, c] = bv[h * DV + (p % DV)]

    # exp-window tiles, indexed so that for j = 128*jc + p, attn[p, i] needs
    # expE[1023 + i - j] = win[p, h, (896 - 128*jc) + i]
    E = (np.asarray(pos_emb, dtype=np.float64) / SCALE)  # [N, HEADS]
    d = np.abs(np.arange(2047) - 1023)
    expE = np.exp(E[d, :])  # [2047, HEADS] float64
    idx = np.arange(WIN)[None, :] - np.arange(128)[:, None] + 127  # [128, WIN]
    win = expE[idx, :].transpose(0, 2, 1)  # [128, HEADS, WIN]
    win = np.ascontiguousarray(win).astype(BF16)

    shared = dict(wqs=wqs, wks=wks, wvs=wvs, wos=wos,
                  bqs=bqs, bks=bks, bos=bos, bvg=bvg, win=win)
    return shared


def _x_shard(x, i):
    # device consumes x as bf16 [128, 2, n] (channel chunks c = a*128 + p)
    xi = np.asarray(x[i], dtype=np.float32).reshape(2, 128, N).transpose(1, 0, 2)
    return np.ascontiguousarray(xi).astype(BF16)


def _build_nc():
    import concourse.bass as bass
    import concourse.mybir as mybir
    import concourse.tile as tile
    from concourse import bacc

    f32 = mybir.dt.float32
    bf16 = mybir.dt.bfloat16

    nc = bacc.Bacc(None, target_bir_lowering=False)

    x_ext = nc.declare_dram_parameter("x", [128, 2, N], bf16, isOutput=False)
    wqs_ext = nc.declare_dram_parameter("wqs", [128, 2, IDK], bf16, isOutput=False)
    wks_ext = nc.declare_dram_parameter("wks", [128, 2, IDK], bf16, isOutput=False)
    wvs_ext = nc.declare_dram_parameter("wvs", [128, 2, IDV], bf16, isOutput=False)
    wos_ext = nc.declare_dram_parameter("wos", [128, 4, DIM_OUT], bf16, isOutput=False)
    bqs_ext = nc.declare_dram_parameter("bqs", [128, 2], f32, isOutput=False)
    bks_ext = nc.declare_dram_parameter("bks", [128, 2], f32, isOutput=False)
    bos_ext = nc.declare_dram_parameter("bos", [128, 2], f32, isOutput=False)
    bvg_ext = nc.declare_dram_parameter("bvg", [128, 4], f32, isOutput=False)
    win_ext = nc.declare_dram_parameter("win", [128, HEADS, WIN], bf16, isOutput=False)
    # bf16 output halves the final store (host upcasts); quantization adds
    # ~0.1% rms against a 2e-2 budget
    out_ext = nc.declare_dram_parameter("out", [DIM_OUT, N], bf16, isOutput=True)

    Exp = mybir.ActivationFunctionType.Exp
    Gelu = mybir.ActivationFunctionType.Gelu
    Identity = mybir.ActivationFunctionType.Identity

    with tile.TileContext(nc) as tc:
        with (
            tc.tile_pool(name="consts", bufs=1) as consts,
            tc.tile_pool(name="scratch", bufs=2) as scratch,
            tc.tile_pool(name="attnp", bufs=12) as attnp,
            tc.tile_pool(name="normp", bufs=6) as normp,
            tc.tile_pool(name="psum", bufs=2, space="PSUM") as psum,
        ):
            # ---- PE warm-up: the HAM clock gate only opens (1.2->2.4GHz)
            # after ~3.4us of sustained matmul activity; dummy matmuls during
            # the DMA wait buy the fast clock for the real stream ----
            dummy = consts.tile([128, 512], bf16)
            nc.vector.memset(dummy, 0.0)
            warm_ps = psum.tile([128, N], f32, tag="dots", bufs=2, name="warmps")
            for _ in range(12):
                nc.tensor.matmul(warm_ps[:, 0:512], lhsT=dummy[:, 0:128],
                                 rhs=dummy, start=True, stop=True)

            # ---- constant loads, split across three DMA queues: SP carries
            # the q/k critical path, ACT the small biases, GpSimd the bulk ----
            xb = consts.tile([128, 2, N], bf16)
            wq = consts.tile([128, 2, IDK], bf16)
            wk = consts.tile([128, 2, IDK], bf16)
            wv = consts.tile([128, 2, IDV], bf16)
            wo = consts.tile([128, 4, DIM_OUT], bf16)
            bqs = consts.tile([128, 2], f32)
            bks = consts.tile([128, 2], f32)
            bos = consts.tile([128, 2], f32)
            bvg_dma = consts.tile([128, 4], f32)
            bvg = consts.tile([128, 4], f32)
            win = consts.tile([128, HEADS, WIN], bf16)

            # the critical q/k path owns the SP queue in issue order (issue
            # order serializes the transfers, so the bulk win load cannot
            # steal HBM bandwidth from x/wq/wk); win rides behind it in
            # column chunks ordered to match ascending-jc consumption --
            # window lateness only delays attn@v (PE slack), never the exps
            # x split by column half: the ic=0 projections need only the left
            # 512 columns of both kc chunks, so the first dots fires ~3us
            # earlier than with whole-tensor loads
            nc.sync.dma_start(out=wq, in_=wqs_ext[:])
            nc.sync.dma_start(out=xb[:, :, 0:512], in_=x_ext[:, :, 0:512])
            nc.sync.dma_start(out=wk, in_=wks_ext[:])
            nc.sync.dma_start(out=xb[:, :, 512:1024], in_=x_ext[:, :, 512:1024])
            nc.sync.dma_start(out=win[:, :, 896:1408], in_=win_ext[:, :, 896:1408])
            nc.sync.dma_start(out=win[:, :, 1408:1920], in_=win_ext[:, :, 1408:1920])
            nc.sync.dma_start(out=win[:, :, 384:896], in_=win_ext[:, :, 384:896])
            nc.sync.dma_start(out=win[:, :, 0:384], in_=win_ext[:, :, 0:384])

            nc.scalar.dma_start(out=bqs, in_=bqs_ext[:])
            nc.scalar.dma_start(out=bks, in_=bks_ext[:])

            nc.gpsimd.dma_start(out=wv, in_=wvs_ext[:])
            nc.gpsimd.dma_start(out=wo, in_=wos_ext[:])
            nc.gpsimd.dma_start(out=bos, in_=bos_ext[:])
            nc.gpsimd.dma_start(out=bvg_dma, in_=bvg_ext[:])

            # dummy exp: walrus inserts the exp table load before ACT's first
            # Exp -- placing one here pulls the ~2.7us load into the DMA wait
            warm = scratch.tile([1, 8], f32, tag="warm")
            nc.vector.memset(warm, 1.0)
            nc.scalar.activation(warm, warm, Exp)

            # ---- persistent intermediates ----
            q_sb = consts.tile([128, 2, N], bf16)   # [ (h,d) chunks, i ]
            k_sb = consts.tile([128, 2, N], bf16)   # [ (h,d) chunks, j ]
            # v columns 64:128 per head; columns 0:64 all-ones so the softmax
            # sums land 64-replicated on partitions 0:64 of the attn@v psum
            # (direct reciprocal input, no SBUF->SBUF widen).  No bias column:
            # bv is applied by the gelu.
            v_aug = consts.tile([128, 8, HEADS, 128], bf16)  # [j-part, jc, h, one|dv]
            g_sb = consts.tile([128, 4, N], bf16)   # gelu input/output [(h,dv) chunks, i]

            # ---- q/k projections; mc=0/ic=0 evacuated on the (idle) ACT
            # engine since it gates the first dots, the rest later on DVE ----
            def emit_qk(mc, evac, ics=(0, 1)):
                for ic in ics:
                    for (w_t, b_t, dst) in ((wq, bqs, q_sb), (wk, bks, k_sb)):
                        ps = psum.tile([128, 512], f32, tag="ops", bufs=4,
                                       name=f"qkps_{mc}_{ic}")
                        for kc in range(2):
                            nc.tensor.matmul(
                                ps,
                                lhsT=w_t[:, kc, mc * 128:(mc + 1) * 128],
                                rhs=xb[:, kc, ic * 512:(ic + 1) * 512],
                                start=(kc == 0), stop=(kc == 1))
                        if evac == "act":
                            nc.scalar.activation(
                                dst[:, mc, ic * 512:(ic + 1) * 512], ps,
                                Identity, bias=b_t[:, mc:mc + 1])
                        else:
                            nc.vector.tensor_scalar_add(
                                dst[:, mc, ic * 512:(ic + 1) * 512], ps,
                                b_t[:, mc:mc + 1])

            nc.vector.memset(v_aug[:, :, :, 0:DV], 1.0)

            def emit_vproj(jc):
                ps = psum.tile([128, 512], f32, tag="ops", bufs=4,
                               name=f"vps_{jc}")
                for kc in range(2):
                    nc.tensor.matmul(
                        ps,
                        lhsT=xb[:, kc, jc * 128:(jc + 1) * 128],
                        rhs=wv[:, kc, :],
                        start=(kc == 0), stop=(kc == 1))
                nc.vector.tensor_copy(
                    v_aug[:, jc, :, DV:128],
                    ps.rearrange("p (h d) -> p h d", h=HEADS))

            emit_qk(0, "act", ics=(0,))
            # v-projections for the left x half run in the pre-dots window
            # (PE would otherwise idle waiting for the q/k evacuations)
            emit_vproj(0)
            emit_vproj(1)
            emit_vproj(2)
            emit_vproj(3)

            # ---- attention: head pairs; the two heads' K=32 dots matmuls
            # share the PE array via distinct 32-row groups; two dots tiles ->
            # one exp + one pair-strided window multiply; attn@v lags ~3 units
            # mid-pair and fully drains (plus both heads' norm chains) at each
            # pair's last unit so the next pair's accumulators start clean ----
            import concourse.bass as bass_mod
            units = [(p, jc) for p in range(4) for jc in range(8)]
            pair_state = {}
            attnv_q = []
            attn_tiles = {}

            def emit_attnv(p, jc, at, ic_major=False):
                if p not in pair_state:
                    pair_state[p] = [[psum.tile([128, 512], f32, tag="ops",
                                                bufs=4, name=f"ops_{h}_{ic}")
                                      for ic in range(2)] for h in (2 * p, 2 * p + 1)]
                st = pair_state[p]
                order = [(hh, ic) for ic in range(2) for hh in range(2)] \
                    if ic_major else [(hh, ic) for hh in range(2) for ic in range(2)]
                for hh, ic in order:
                    nc.tensor.matmul(
                        st[hh][ic], lhsT=v_aug[:, jc, 2 * p + hh, :],
                        rhs=at[:, ic, hh, :],
                        start=(jc == 0), stop=(jc == 7))

            def emit_norm_chain(h, ic, ops):
                # g[dv, i] = out_u[dv, i] / sums[i]; sums 64-replicated on
                # partitions 0:64, out_u on 64:128
                bc = normp.tile([DV, 512], f32, tag="bc", name=f"bc_{h}_{ic}")
                nc.vector.reciprocal_approx_fast(bc, ops[0:DV, :])
                nc.vector.tensor_mul(
                    g_sb[(h % 2) * DV:(h % 2) * DV + DV, h // 2,
                         ic * 512:(ic + 1) * 512],
                    ops[DV:128, :], bc)

            def drain_to(n):
                while len(attnv_q) > n:
                    emit_attnv(*attnv_q.pop(0))

            pending_norms = []
            dps_tiles = {}
            for ui, (p, jc) in enumerate(units):
                h0, h1 = 2 * p, 2 * p + 1
                koff0, kch0 = (h0 % 4) * 32, h0 // 4
                koff1, kch1 = (h1 % 4) * 32, h1 // 4
                off = 896 - 128 * jc
                # attn layout: [128, (ic, head-half, 512)]
                attn = attnp.tile([128, 2, 2, 512], bf16, tag="attn",
                                  name=f"attn_{p}_{jc}")
                for ic in range(2):
                    dps = psum.tile([128, N], f32, tag="dots", bufs=2,
                                    name=f"dots_{p}_{jc}_{ic}")
                    dps_tiles[(ui, ic)] = dps
                    nc.tensor.matmul(
                        dps[:, 0:512],
                        lhsT=k_sb[koff0:koff0 + 32, kch0, jc * 128:(jc + 1) * 128],
                        rhs=q_sb[koff0:koff0 + 32, kch0, ic * 512:(ic + 1) * 512],
                        start=True, stop=True, tile_position=(koff0, 0))
                    nc.tensor.matmul(
                        dps[:, 512:1024],
                        lhsT=k_sb[koff1:koff1 + 32, kch1, jc * 128:(jc + 1) * 128],
                        rhs=q_sb[koff1:koff1 + 32, kch1, ic * 512:(ic + 1) * 512],
                        start=True, stop=True, tile_position=(koff1, 0))
                    nc.scalar.activation(attn[:, ic, :, :], dps, Exp)
                    if ui == 0 and ic == 0:
                        # ic=1 projections emitted between the first unit's
                        # halves: the ic=1 dots right after must not precede
                        # their producers in the in-order PE queue, and the
                        # ic=0 dots must not wait on the right x half
                        emit_qk(0, "dve", ics=(1,))
                # one window multiply for the whole (pair, jc) tile; the very
                # last unit splits it per-ic so the closing attn@v + norm +
                # gelu + out-proj chain starts half an exp earlier
                if ui == 31:
                    for ic in range(2):
                        wv_v = win[:, h0, off + 512 * ic:off + 512 * ic + 512]
                        wv_half = bass_mod.AP(
                            tensor=wv_v.tensor, offset=wv_v.offset,
                            ap=[list(wv_v.ap[0]), [WIN, 2], [1, 512]])
                        nc.vector.tensor_mul(attn[:, ic, :, :],
                                             attn[:, ic, :, :], wv_half)
                else:
                    wv_view = win[:, h0, off:off + 512]
                    wv_quad = bass_mod.AP(
                        tensor=wv_view.tensor, offset=wv_view.offset,
                        ap=[list(wv_view.ap[0]), [512, 2], [WIN, 2], [1, 512]])
                    nc.vector.tensor_mul(attn, attn, wv_quad)
                attnv_q.append((p, jc, attn))
                attn_tiles[ui] = attn
                # interleave the remaining setup work behind the first units
                # (their PSUM-pool slots must rotate before the first pair's
                # accumulators claim the tag)
                if ui == 0:
                    emit_vproj(4)
                    emit_vproj(5)
                elif ui == 1:
                    emit_vproj(6)
                    emit_vproj(7)
                elif ui == 2:
                    emit_qk(1, "dve")
                # attn@v pop schedule: steady lag 2 mid-pair (one 4-matmul
                # pop per unit; pair 0 starts a little later so the window
                # DMA chunks always beat their first consumer), with the
                # previous pair fully drained at the next pair's first unit
                # so its norm chains can ride units jc0/jc1 -- the
                # accumulators are then free ~1 unit before the new pair's
                # first attn@v claims the PSUM slots
                if jc == 0 and p > 0:
                    drain_to(1)
                    st = pair_state.pop(p - 1)
                    pending_norms = [(h0 - 2, 0, st[0][0]), (h0 - 1, 0, st[1][0]),
                                     (h0 - 2, 1, st[0][1]), (h0 - 1, 1, st[1][1])]
                    for args in pending_norms[:2]:
                        emit_norm_chain(*args)
                    pending_norms = pending_norms[2:]
                else:
                    drain_to(2)
                    if jc == 1 and pending_norms:
                        for args in pending_norms:
                            emit_norm_chain(*args)
                        pending_norms = []
                # filler matmuls into the previous unit's consumed dots banks
                # (anchored on the same exp the next dots waits on, so zero
                # added dependency latency): real PE work in the ACT-bound
                # idle slivers keeps the HAM activity monitor from
                # re-throttling the clock -- the setup-heavy early units
                # demonstrably cross two pairs without a single re-throttle
                if ui >= 1:
                    for fic in range(2):
                        nc.tensor.matmul(
                            dps_tiles[(ui - 1, fic)][:, 0:512],
                            lhsT=dummy[:, 0:128], rhs=dummy,
                            start=True, stop=True)
            while attnv_q:
                emit_attnv(*attnv_q.pop(0), ic_major=True)
            # the bvg copy reads the last unit's attn tile, so the scheduler
            # cannot hoist the gelus (which bias off bvg) into the exp stream
            # -- a mid-stream gelu costs two ~2.7us ACT table-set switches
            nc.vector.scalar_tensor_tensor(
                bvg, in0=attn_tiles[31][:, 0, 0, 0:4], scalar=0.0,
                in1=bvg_dma, op0=mybir.AluOpType.mult, op1=mybir.AluOpType.add)
            # pair 3's norm chains, ic0 first: the ic0 half of the final gelu
            # + out-proj + store pipeline starts while ic1 still normalizes
            st = pair_state.pop(3)
            for ic in range(2):
                for hh in range(2):
                    emit_norm_chain(6 + hh, ic, st[hh][ic])

            # ---- tail: gelu per chunk (bias = folded v-bias) gates its
            # out-proj kc round; table load overlaps the last norm chains ----
            out_r = out_ext[:].rearrange("(a p) n -> p a n", p=128)
            fps_t = {}
            for ic in range(2):
                for mc in range(2):
                    fps_t[(mc, ic)] = psum.tile([128, 512], f32, tag="ops",
                                                bufs=4, name=f"fin_{mc}_{ic}")
            for kc in range(3):
                gch = g_sb[:, kc, :]
                nc.scalar.activation(gch, gch, Gelu, bias=bvg[:, kc:kc + 1])
                for ic in range(2):
                    for mc in range(2):
                        nc.tensor.matmul(
                            fps_t[(mc, ic)],
                            lhsT=wo[:, kc, mc * 128:(mc + 1) * 128],
                            rhs=g_sb[:, kc, ic * 512:(ic + 1) * 512],
                            start=(kc == 0), stop=False)
            for ic in range(2):
                g3 = g_sb[:, 3, ic * 512:(ic + 1) * 512]
                nc.scalar.activation(g3, g3, Gelu, bias=bvg[:, 3:4])
                for mc in range(2):
                    nc.tensor.matmul(
                        fps_t[(mc, ic)],
                        lhsT=wo[:, 3, mc * 128:(mc + 1) * 128],
                        rhs=g3, start=False, stop=True)
                for mc in range(2):
                    o_sb = scratch.tile([128, 512], bf16, tag="osb",
                                        bufs=4, name=f"osb_{mc}_{ic}")
                    if mc == 0:
                        nc.scalar.activation(o_sb, fps_t[(mc, ic)], Identity,
                                             bias=bos[:, mc:mc + 1])
                        nc.scalar.dma_start(
                            out=out_r[:, mc, ic * 512:(ic + 1) * 512], in_=o_sb)
                    else:
                        nc.vector.tensor_scalar_add(o_sb, fps_t[(mc, ic)],
                                                    bos[:, mc:mc + 1])
                        nc.sync.dma_start(
                            out=out_r[:, mc, ic * 512:(ic + 1) * 512], in_=o_sb)

    nc.finalize()
    return nc


_NC_CACHE = None


def kernel(**inputs) -> np.ndarray:
    global _NC_CACHE
    from concourse.bass_utils import run_bass_kernel_spmd

    x = np.asarray(inputs["x"], dtype=np.float32)
    shared = _prep_host(**inputs)

    if _NC_CACHE is None:
        _NC_CACHE = _build_nc()
    nc = _NC_CACHE

    in_maps = [dict(x=_x_shard(x, i), **shared) for i in range(B)]
    res = run_bass_kernel_spmd(nc, in_maps, core_ids=list(range(B)))
    out = np.stack([res.results[i]["out"] for i in range(B)], axis=0)
    return out.astype(np.float32)
